# revision 15
# baseline (speedup 1.0000x reference)
"""BitLinear (ternary-quantized linear) Trainium2 kernel.

Computes y = x @ w_q^T where w_q = clip(round(w/(alpha+eps)), -1, 1) * alpha
and alpha = mean(|w|) over the FULL weight.

Distribution (8 NeuronCores, tensor-parallel):
  - weight rows (out_features) sharded 8 ways: each core owns N = 16384/8 = 2048
    output features.
  - x replicated to every core (pre-transposed + bf16 on host for layout).
  - alpha needs the global |w| mean: each core reduces its own shard, then an
    on-device AllReduce across the 8 cores produces the global sum.
  - outputs stay column-sharded; host concatenates the 8 shards.

Per-core device kernel:
  phase 1: abs-sum reduce of w shard (DVE reduce w/ apply_absolute_value),
           cross-partition sum via PE ones-matmul, AllReduce, then
           alpha = S * 2^-26, r = 1/(alpha+eps) (DVE reciprocal).
  phase 2: quantize w -> t in {-1,0,+1} as bf16 (exact), using the fp32
           round-to-nearest-even "magic number" trick: RNE(z) = (z+1.5*2^23)-1.5*2^23.
           t kept entirely in SBUF ([128, 32, 2048] bf16 = 128KB/partition).
  phase 3: y = x @ t^T on the PE in bf16 (fp32 PSUM accumulation), tiled
           [128m x 128k x 512n]; PSUM evicted with a fused *alpha scale on DVE.
"""

import numpy as np
import ml_dtypes

import concourse.bass as bass
import concourse.mybir as mybir
import concourse.tile as tile
from concourse import bacc
from concourse.bass_utils import run_bass_kernel_spmd

P = 128
N_CORES = 8

# Full problem shapes (hardcoded per contract).
B, S, K = 4, 2048, 4096
N_FULL = 16384
M = B * S                      # 8192 rows of x
N = N_FULL // N_CORES          # 2048 output features per core

MAGIC = 12582912.0             # 1.5 * 2**23: fp32 RNE rounding constant
EPS = 1e-8
MEAN_SCALE = float(2.0 ** -26)  # 1 / (16384*4096), exact power of two

F32 = mybir.dt.float32
BF16 = mybir.dt.bfloat16


def build_nc(M, K, N, n_cores=N_CORES, mc=128, qw=2048):
    """Build the per-core Bass program. All cores run the same program (SPMD)."""
    KO = K // P          # k tiles of 128 partitions
    NT = N // 512        # matmul n tiles (psum banks per m-subtile)
    NQ = max(N // qw, 1) # quantization column tiles per k tile
    qw = min(qw, N)
    MCH = M // mc        # m chunks
    MS = mc // P         # m subtiles per chunk

    nc = bacc.Bacc("TRN2", target_bir_lowering=False, debug=False,
                   num_devices=n_cores)

    # xt is pre-tiled on host: [MCH, 128, KO*mc] so each m-chunk is one fully
    # contiguous DMA (16KB per partition row).
    xt = nc.dram_tensor("xt", [MCH, P, KO * mc], BF16, kind="ExternalInput")
    wt = nc.dram_tensor("wt", [K, N], F32, kind="ExternalInput")
    y = nc.dram_tensor("y", [M, N], F32, kind="ExternalOutput")
    cc_in = nc.dram_tensor("cc_in", [1, 1], F32)
    cc_out = nc.dram_tensor("cc_out", [1, 1], F32, addr_space="Shared")

    w_view = wt.ap().rearrange("(ko p) n -> ko p n", p=P)    # [KO, 128, N]
    y_view = y.ap().rearrange("(mo p) n -> mo p n", p=P)     # [M/128, 128, N]

    with tile.TileContext(nc) as tc:
        with (
            tc.tile_pool(name="const", bufs=1) as const,
            tc.tile_pool(name="scal", bufs=1) as scal,
            tc.tile_pool(name="wstage", bufs=3) as wstage,
            tc.tile_pool(name="tpool", bufs=1) as tpool,
            tc.tile_pool(name="xstage", bufs=2) as xstage,
            tc.tile_pool(name="outp", bufs=2) as outp,
            tc.tile_pool(name="psum", bufs=1, space="PSUM") as psum,
        ):
            # ---------- phase 1: global alpha ----------
            # |w| row-sums split between DVE (tensor_reduce w/ abs) and ACT
            # (activation Abs with accum_out) so the reduce tail is half as long.
            acc = scal.tile([P, KO * NQ], F32)
            for kq in range(KO):
                for nq in range(NQ):
                    idx = kq * NQ + nq
                    par = idx % 2
                    w_t = wstage.tile([P, qw], F32, tag=f"wst{par}",
                                      bufs=3 if par == 0 else 2,
                                      name=f"wr_{kq}_{nq}")
                    nc.sync.dma_start(
                        out=w_t, in_=w_view[kq, :, nq * qw:(nq + 1) * qw])
                    if par == 0:
                        nc.vector.tensor_reduce(
                            out=acc[:, idx:idx + 1], in_=w_t,
                            axis=mybir.AxisListType.X, op=mybir.AluOpType.add,
                            apply_absolute_value=True)
                    else:
                        nc.scalar.activation(
                            out=w_t, in_=w_t,
                            func=mybir.ActivationFunctionType.Abs,
                            accum_out=acc[:, idx:idx + 1])

            s_p = scal.tile([P, 1], F32)
            nc.vector.reduce_sum(out=s_p, in_=acc, axis=mybir.AxisListType.X)
            ones = const.tile([P, 1], F32)
            nc.vector.memset(ones, 1.0)
            ps_s = psum.tile([1, 1], F32, tag="mm0", bufs=2, name="ps_s")
            nc.tensor.matmul(ps_s, lhsT=ones, rhs=s_p, start=True, stop=True)
            s_all = scal.tile([1, 1], F32)
            nc.vector.tensor_copy(out=s_all, in_=ps_s)
            nc.sync.dma_start(out=cc_in.ap(), in_=s_all)
            nc.gpsimd.collective_compute(
                "AllReduce", mybir.AluOpType.add,
                replica_groups=[list(range(n_cores))],
                ins=[cc_in.ap()], outs=[cc_out.ap()])

            s_b = scal.tile([P, 1], F32)
            nc.sync.dma_start(out=s_b, in_=cc_out.ap().to_broadcast([P, 1]))
            alpha = scal.tile([P, 1], F32)
            nc.vector.tensor_scalar_mul(alpha, s_b, MEAN_SCALE)
            aeps = scal.tile([P, 1], F32)
            nc.vector.tensor_scalar_add(aeps, alpha, EPS)
            r = scal.tile([P, 1], F32)
            nc.vector.reciprocal(r, aeps)

            # ---------- phase 2: quantize w -> t (bf16 ternary) ----------
            # DVE: z = RNE(w*r) + MAGIC (dual op mult+add), then clamp in
            # magic space to [MAGIC-1, MAGIC+1] (dual op min+max).
            # ACT: subtract MAGIC and cast to bf16 (Copy with bias=-MAGIC).
            t = tpool.tile([P, KO, N], BF16)
            for kq in range(KO):
                for nq in range(NQ):
                    idx = kq * NQ + nq
                    par = idx % 2
                    w_t = wstage.tile([P, qw], F32, tag=f"wst{par}",
                                      bufs=3 if par == 0 else 2,
                                      name=f"wq_{kq}_{nq}")
                    nc.sync.dma_start(
                        out=w_t, in_=w_view[kq, :, nq * qw:(nq + 1) * qw])
                    if par == 0:
                        nc.vector.tensor_scalar(out=w_t, in0=w_t,
                                                scalar1=r[:, :],
                                                scalar2=MAGIC,
                                                op0=mybir.AluOpType.mult,
                                                op1=mybir.AluOpType.add)
                    else:
                        nc.scalar.activation(
                            out=w_t, in_=w_t,
                            func=mybir.ActivationFunctionType.Copy,
                            bias=MAGIC, scale=r[:, :])
                    nc.vector.tensor_scalar(out=w_t, in0=w_t,
                                            scalar1=MAGIC + 1.0,
                                            scalar2=MAGIC - 1.0,
                                            op0=mybir.AluOpType.min,
                                            op1=mybir.AluOpType.max)
                    nc.scalar.activation(
                        out=t[:, kq, nq * qw:(nq + 1) * qw], in_=w_t,
                        func=mybir.ActivationFunctionType.Copy,
                        bias=-MAGIC, scale=1.0)

            # ---------- phase 3: y = (x @ t^T) * alpha ----------
            for mch in range(MCH):
                xt_t = xstage.tile([P, KO, mc], BF16, tag="xt", bufs=2,
                                   name=f"xt_{mch}")
                nc.sync.dma_start(
                    out=xt_t,
                    in_=xt.ap()[mch].rearrange("p (ko m) -> p ko m", ko=KO))
                for ms in range(MS):
                    m_idx = mch * MS + ms
                    psums = [
                        psum.tile([P, 512], F32, tag=f"mm{n}", bufs=2,
                                  name=f"ps_{m_idx}_{n}")
                        for n in range(NT)
                    ]
                    for kq in range(KO):
                        lhs = xt_t[:, kq, ms * P:(ms + 1) * P]
                        for n in range(NT):
                            nc.tensor.matmul(
                                psums[n], lhsT=lhs,
                                rhs=t[:, kq, n * 512:(n + 1) * 512],
                                start=(kq == 0), stop=(kq == KO - 1))
                    out_t = outp.tile([P, N], F32, tag="out", bufs=2,
                                      name=f"out_{m_idx}")
                    for n in range(NT):
                        nc.vector.tensor_scalar_mul(
                            out_t[:, n * 512:(n + 1) * 512], psums[n],
                            alpha[:, :])
                    nc.sync.dma_start(out=y_view[m_idx], in_=out_t)

    nc.compile()
    return nc


def prep_xt(x_flat: np.ndarray, mc: int) -> np.ndarray:
    """Pre-tile x for contiguous chunk DMA:
    xt[mch, p, ko*mc+m] = bf16(x_flat[mch*mc+m, ko*128+p])."""
    m, k = x_flat.shape
    mch, ko = m // mc, k // P
    return np.ascontiguousarray(
        x_flat.reshape(mch, mc, ko, P).astype(ml_dtypes.bfloat16)
        .transpose(0, 3, 2, 1)).reshape(mch, P, ko * mc)


def kernel(x: np.ndarray, weight: np.ndarray) -> np.ndarray:
    assert x.shape == (B, S, K) and weight.shape == (N_FULL, K)

    # host-side layout prep (no math beyond the bf16 cast of x)
    xt = prep_xt(x.reshape(M, K), 128)
    in_maps = []
    for c in range(N_CORES):
        wt_c = np.ascontiguousarray(
            weight[c * N:(c + 1) * N, :].T.astype(np.float32))  # [K, N] f32
        in_maps.append({"xt": xt, "wt": wt_c})

    nc = build_nc(M, K, N)
    res = run_bass_kernel_spmd(nc, in_maps, list(range(N_CORES)))
    y = np.concatenate([res.results[c]["y"] for c in range(N_CORES)], axis=1)
    return np.ascontiguousarray(y.reshape(B, S, N_FULL).astype(np.float32))


# revision 16
# speedup vs baseline: 1.1727x; 1.1727x over previous
"""BitLinear (ternary-quantized linear) Trainium2 kernel.

Computes y = x @ w_q^T where w_q = clip(round(w/(alpha+eps)), -1, 1) * alpha
and alpha = mean(|w|) over the FULL weight.

Distribution (8 NeuronCores, tensor-parallel):
  - weight rows (out_features) sharded 8 ways: each core owns N = 16384/8 = 2048
    output features.
  - x replicated to every core (pre-tiled + bf16 on host for DMA layout).
  - alpha needs the global |w| mean: launch 1 reduces each shard on its core
    (DVE/ACT abs-row-sums + PE ones-matmul cross-partition sum); the 8 partial
    sums are all-reduced across shards on the host (8 scalar adds). An
    on-device ncfw AllReduce was measured to derate ALL subsequent PE work by
    ~25% for the rest of the NEFF, so the collective is deliberately kept out
    of the main launch.
  - outputs stay column-sharded; host concatenates the 8 shards.

Launch 2 per-core kernel:
  quantize: t = clip(RNE(w*r), -1, 1) in {-1,0,+1} as bf16 (exact), via the
           fp32 round-to-nearest-even "magic number" trick with the clamp
           fused in magic space:
             DVE:  u = (w*r) + 1.5*2^23            (dual op mult+add)
             DVE:  u = min(max(u, M-1), M+1)       (dual op min+max)
             ACT:  t = u - 1.5*2^23 -> bf16        (Copy with bias=-MAGIC)
           t kept entirely in SBUF ([128, 32, 2048] bf16 = 128KB/partition).
  matmul:  y = x @ t^T on the PE in bf16 (fp32 PSUM accumulation), tiled
           [128m x 128k x 512n], 4 PSUM banks per m-subtile, double-buffered;
           PSUM evicted with a fused *alpha scale on DVE; alpha/r arrive as a
           tiny [1,2] input, partition-broadcast by DMA.
"""

import numpy as np
import ml_dtypes

import concourse.bass as bass
import concourse.mybir as mybir
import concourse.tile as tile
from concourse import bacc
from concourse.bass_utils import run_bass_kernel_spmd

P = 128
N_CORES = 8

# Full problem shapes (hardcoded per contract).
B, S, K = 4, 2048, 4096
N_FULL = 16384
M = B * S                      # 8192 rows of x
N = N_FULL // N_CORES          # 2048 output features per core

MAGIC = 12582912.0             # 1.5 * 2**23: fp32 RNE rounding constant
EPS = 1e-8
MEAN_SCALE = float(2.0 ** -26)  # 1 / (16384*4096), exact power of two

F32 = mybir.dt.float32
BF16 = mybir.dt.bfloat16


def build_reduce_nc(K, N, n_cores=N_CORES, qw=2048):
    """Launch 1: per-core sum(|w_shard|) -> scalar output "s"."""
    KO = K // P
    NQ = max(N // qw, 1)
    qw = min(qw, N)

    nc = bacc.Bacc("TRN2", target_bir_lowering=False, debug=False,
                   num_devices=n_cores)
    wt = nc.dram_tensor("wt", [K, N], F32, kind="ExternalInput")
    s_out = nc.dram_tensor("s", [1, 1], F32, kind="ExternalOutput")
    w_view = wt.ap().rearrange("(ko p) n -> ko p n", p=P)

    with tile.TileContext(nc) as tc:
        with (
            tc.tile_pool(name="const", bufs=1) as const,
            tc.tile_pool(name="scal", bufs=1) as scal,
            tc.tile_pool(name="wstage", bufs=4) as wstage,
            tc.tile_pool(name="psum", bufs=1, space="PSUM") as psum,
        ):
            acc = scal.tile([P, KO * NQ], F32)
            for kq in range(KO):
                for nq in range(NQ):
                    idx = kq * NQ + nq
                    par = idx % 2
                    w_t = wstage.tile([P, qw], F32, tag=f"wst{par}", bufs=4,
                                      name=f"wr_{kq}_{nq}")
                    nc.sync.dma_start(
                        out=w_t, in_=w_view[kq, :, nq * qw:(nq + 1) * qw])
                    if par == 0:
                        nc.vector.tensor_reduce(
                            out=acc[:, idx:idx + 1], in_=w_t,
                            axis=mybir.AxisListType.X, op=mybir.AluOpType.add,
                            apply_absolute_value=True)
                    else:
                        nc.scalar.activation(
                            out=w_t, in_=w_t,
                            func=mybir.ActivationFunctionType.Abs,
                            accum_out=acc[:, idx:idx + 1])

            s_p = scal.tile([P, 1], F32)
            nc.vector.reduce_sum(out=s_p, in_=acc, axis=mybir.AxisListType.X)
            ones = const.tile([P, 1], F32)
            nc.vector.memset(ones, 1.0)
            ps_s = psum.tile([1, 1], F32, tag="mm0", name="ps_s")
            nc.tensor.matmul(ps_s, lhsT=ones, rhs=s_p, start=True, stop=True)
            s_all = scal.tile([1, 1], F32)
            nc.vector.tensor_copy(out=s_all, in_=ps_s)
            nc.sync.dma_start(out=s_out.ap(), in_=s_all)

    nc.compile()
    return nc


def build_main_nc(M, K, N, n_cores=N_CORES, mc=128, qw=2048):
    """Launch 2: quantize + matmul. alpha/r arrive via the "sc" input."""
    KO = K // P          # k tiles of 128 partitions
    NT = N // 512        # matmul n tiles (psum banks per m-subtile)
    NQ = max(N // qw, 1)
    qw = min(qw, N)
    MCH = M // mc        # m chunks
    MS = mc // P         # m subtiles per chunk

    nc = bacc.Bacc("TRN2", target_bir_lowering=False, debug=False,
                   num_devices=n_cores)

    # xt is pre-tiled on host: [MCH, 128, KO*mc] so each m-chunk is one fully
    # contiguous DMA.
    xt = nc.dram_tensor("xt", [MCH, P, KO * mc], BF16, kind="ExternalInput")
    wt = nc.dram_tensor("wt", [K, N], F32, kind="ExternalInput")
    sc = nc.dram_tensor("sc", [1, 2], F32, kind="ExternalInput")  # [alpha, r]
    y = nc.dram_tensor("y", [M, N], F32, kind="ExternalOutput")

    w_view = wt.ap().rearrange("(ko p) n -> ko p n", p=P)    # [KO, 128, N]
    y_view = y.ap().rearrange("(mo p) n -> mo p n", p=P)     # [M/128, 128, N]

    with tile.TileContext(nc) as tc:
        with (
            tc.tile_pool(name="scal", bufs=1) as scal,
            tc.tile_pool(name="wstage", bufs=3) as wstage,
            tc.tile_pool(name="tpool", bufs=1) as tpool,
            tc.tile_pool(name="xstage", bufs=2) as xstage,
            tc.tile_pool(name="outp", bufs=2) as outp,
            tc.tile_pool(name="psum", bufs=1, space="PSUM") as psum,
        ):
            sc_b = scal.tile([P, 2], F32)
            nc.sync.dma_start(out=sc_b, in_=sc.ap().to_broadcast([P, 2]))
            alpha = sc_b[:, 0:1]
            r = sc_b[:, 1:2]

            # ---------- quantize w -> t (bf16 ternary) ----------
            t = tpool.tile([P, KO, N], BF16)
            for kq in range(KO):
                for nq in range(NQ):
                    idx = kq * NQ + nq
                    par = idx % 2
                    w_t = wstage.tile([P, qw], F32, tag=f"wst{par}",
                                      bufs=3 if par == 0 else 2,
                                      name=f"wq_{kq}_{nq}")
                    nc.sync.dma_start(
                        out=w_t, in_=w_view[kq, :, nq * qw:(nq + 1) * qw])
                    if par == 0:
                        nc.vector.tensor_scalar(out=w_t, in0=w_t,
                                                scalar1=r,
                                                scalar2=MAGIC,
                                                op0=mybir.AluOpType.mult,
                                                op1=mybir.AluOpType.add)
                    else:
                        nc.scalar.activation(
                            out=w_t, in_=w_t,
                            func=mybir.ActivationFunctionType.Copy,
                            bias=MAGIC, scale=r)
                    nc.vector.tensor_scalar(out=w_t, in0=w_t,
                                            scalar1=MAGIC + 1.0,
                                            scalar2=MAGIC - 1.0,
                                            op0=mybir.AluOpType.min,
                                            op1=mybir.AluOpType.max)
                    nc.scalar.activation(
                        out=t[:, kq, nq * qw:(nq + 1) * qw], in_=w_t,
                        func=mybir.ActivationFunctionType.Copy,
                        bias=-MAGIC, scale=1.0)

            # ---------- y = (x @ t^T) * alpha ----------
            for mch in range(MCH):
                xt_t = xstage.tile([P, KO, mc], BF16, tag="xt", bufs=2,
                                   name=f"xt_{mch}")
                nc.sync.dma_start(
                    out=xt_t,
                    in_=xt.ap()[mch].rearrange("p (ko m) -> p ko m", ko=KO))
                for ms in range(MS):
                    m_idx = mch * MS + ms
                    psums = [
                        psum.tile([P, 512], F32, tag=f"mm{n}", bufs=2,
                                  name=f"ps_{m_idx}_{n}")
                        for n in range(NT)
                    ]
                    for kq in range(KO):
                        lhs = xt_t[:, kq, ms * P:(ms + 1) * P]
                        for n in range(NT):
                            nc.tensor.matmul(
                                psums[n], lhsT=lhs,
                                rhs=t[:, kq, n * 512:(n + 1) * 512],
                                start=(kq == 0), stop=(kq == KO - 1))
                    out_t = outp.tile([P, N], F32, tag="out", bufs=2,
                                      name=f"out_{m_idx}")
                    for n in range(NT):
                        nc.vector.tensor_scalar_mul(
                            out_t[:, n * 512:(n + 1) * 512], psums[n], alpha)
                    nc.sync.dma_start(out=y_view[m_idx], in_=out_t)

    nc.compile()
    return nc


def prep_xt(x_flat: np.ndarray, mc: int) -> np.ndarray:
    """Pre-tile x for contiguous chunk DMA:
    xt[mch, p, ko*mc+m] = bf16(x_flat[mch*mc+m, ko*128+p])."""
    m, k = x_flat.shape
    mch, ko = m // mc, k // P
    return np.ascontiguousarray(
        x_flat.reshape(mch, mc, ko, P).astype(ml_dtypes.bfloat16)
        .transpose(0, 3, 2, 1)).reshape(mch, P, ko * mc)


def kernel(x: np.ndarray, weight: np.ndarray) -> np.ndarray:
    assert x.shape == (B, S, K) and weight.shape == (N_FULL, K)
    core_ids = list(range(N_CORES))

    wts = [np.ascontiguousarray(weight[c * N:(c + 1) * N, :].T
                                .astype(np.float32)) for c in range(N_CORES)]

    # launch 1: per-shard |w| sums on device
    nc1 = build_reduce_nc(K, N)
    res1 = run_bass_kernel_spmd(nc1, [{"wt": w} for w in wts], core_ids)
    partials = [np.float32(res1.results[c]["s"][0, 0]) for c in range(N_CORES)]

    # the all-reduce across shards: 8 scalar adds
    s_tot = np.float32(0.0)
    for p in partials:
        s_tot = np.float32(s_tot + p)
    alpha = np.float32(s_tot * np.float32(MEAN_SCALE))
    r = np.float32(np.float32(1.0) / np.float32(alpha + np.float32(EPS)))
    sc = np.array([[alpha, r]], dtype=np.float32)

    # launch 2: quantize + matmul
    xt = prep_xt(x.reshape(M, K), 128)
    nc2 = build_main_nc(M, K, N)
    in_maps = [{"xt": xt, "wt": wts[c], "sc": sc} for c in range(N_CORES)]
    res2 = run_bass_kernel_spmd(nc2, in_maps, core_ids)

    y = np.concatenate([res2.results[c]["y"] for c in range(N_CORES)], axis=1)
    return np.ascontiguousarray(y.reshape(B, S, N_FULL).astype(np.float32))


# revision 17
# speedup vs baseline: 23.8245x; 20.3166x over previous
"""BitLinear (ternary-quantized linear) Trainium2 kernel.

Computes y = x @ w_q^T where w_q = clip(round(w/(alpha+eps)), -1, 1) * alpha
and alpha = mean(|w|) over the FULL weight.

Distribution (8 NeuronCores, tensor-parallel):
  - weight rows (out_features) sharded 8 ways: each core owns N = 16384/8 = 2048
    output features.
  - x replicated to every core (pre-tiled + bf16 on host for DMA layout).
  - alpha needs the global |w| mean: launch 1 reduces each shard on its core
    (DVE/ACT abs-row-sums + PE ones-matmul cross-partition sum); the 8 partial
    sums are all-reduced across shards on the host (8 scalar adds). An
    on-device ncfw AllReduce was measured to derate ALL subsequent PE work by
    ~25% for the rest of the NEFF, so the collective is deliberately kept out
    of the main launch.
  - outputs stay column-sharded; host concatenates the 8 shards.

Launch 2 per-core kernel:
  quantize: t = clip(RNE(w*r), -1, 1) in {-1,0,+1} as bf16 (exact), via the
           fp32 round-to-nearest-even "magic number" trick with the clamp
           fused in magic space:
             DVE:  u = (w*r) + 1.5*2^23            (dual op mult+add)
             DVE:  u = min(max(u, M-1), M+1)       (dual op min+max)
             ACT:  t = u - 1.5*2^23 -> bf16        (Copy with bias=-MAGIC)
           t kept entirely in SBUF ([128, 32, 2048] bf16 = 128KB/partition).
  matmul:  y = x @ t^T on the PE in bf16 (fp32 PSUM accumulation), tiled
           [128m x 128k x 512n], 4 PSUM banks per m-subtile, double-buffered;
           PSUM evicted with a fused *alpha scale on DVE; alpha/r arrive as a
           tiny [1,2] input, partition-broadcast by DMA.
"""

import numpy as np
import ml_dtypes

import concourse.bass as bass
import concourse.mybir as mybir
import concourse.tile as tile
from concourse import bacc
from concourse.bass_utils import run_bass_kernel_spmd

P = 128
N_CORES = 8

# Full problem shapes (hardcoded per contract).
B, S, K = 4, 2048, 4096
N_FULL = 16384
M = B * S                      # 8192 rows of x
N = N_FULL // N_CORES          # 2048 output features per core

MAGIC = 12582912.0             # 1.5 * 2**23: fp32 RNE rounding constant
EPS = 1e-8
MEAN_SCALE = float(2.0 ** -26)  # 1 / (16384*4096), exact power of two

F32 = mybir.dt.float32
BF16 = mybir.dt.bfloat16


def build_reduce_nc(K, N, n_cores=N_CORES, qw=2048):
    """Launch 1: per-core sum(|w_shard|) -> scalar output "s"."""
    KO = K // P
    NQ = max(N // qw, 1)
    qw = min(qw, N)

    KP = KO // 2         # pairs of k tiles, loaded 4MB at a time

    nc = bacc.Bacc("TRN2", target_bir_lowering=False, debug=False,
                   num_devices=n_cores)
    wt = nc.dram_tensor("wt", [K, N], F32, kind="ExternalInput")
    s_out = nc.dram_tensor("s", [1, 1], F32, kind="ExternalOutput")
    wp_view = wt.ap().rearrange("(kp j p) n -> kp p j n", p=P, j=2)

    with tile.TileContext(nc) as tc:
        with (
            tc.tile_pool(name="const", bufs=1) as const,
            tc.tile_pool(name="scal", bufs=1) as scal,
            tc.tile_pool(name="wstage", bufs=4) as wstage,
            tc.tile_pool(name="psum", bufs=1, space="PSUM") as psum,
        ):
            assert NQ == 1
            acc = scal.tile([P, KP], F32)
            for kp in range(KP):
                par = kp % 2
                w_t = wstage.tile([P, 2, qw], F32, tag=f"wst{par}", bufs=3,
                                  name=f"wr_{kp}")
                nc.sync.dma_start(out=w_t, in_=wp_view[kp])
                if par == 0:
                    nc.vector.tensor_reduce(
                        out=acc[:, kp:kp + 1], in_=w_t,
                        axis=mybir.AxisListType.XY, op=mybir.AluOpType.add,
                        apply_absolute_value=True)
                else:
                    nc.scalar.activation(
                        out=w_t, in_=w_t,
                        func=mybir.ActivationFunctionType.Abs,
                        accum_out=acc[:, kp:kp + 1])

            s_p = scal.tile([P, 1], F32)
            nc.vector.reduce_sum(out=s_p, in_=acc, axis=mybir.AxisListType.X)
            ones = const.tile([P, 1], F32)
            nc.vector.memset(ones, 1.0)
            ps_s = psum.tile([1, 1], F32, tag="mm0", name="ps_s")
            nc.tensor.matmul(ps_s, lhsT=ones, rhs=s_p, start=True, stop=True)
            s_all = scal.tile([1, 1], F32)
            nc.vector.tensor_copy(out=s_all, in_=ps_s)
            nc.sync.dma_start(out=s_out.ap(), in_=s_all)

    nc.compile()
    return nc


def build_main_nc(M, K, N, n_cores=N_CORES, mc=128, qw=2048):
    """Launch 2: quantize + matmul. alpha/r arrive via the "sc" input."""
    KO = K // P          # k tiles of 128 partitions
    NT = N // 512        # matmul n tiles (psum banks per m-subtile)
    NQ = max(N // qw, 1)
    qw = min(qw, N)
    MCH = M // mc        # m chunks
    MS = mc // P         # m subtiles per chunk

    nc = bacc.Bacc("TRN2", target_bir_lowering=False, debug=False,
                   num_devices=n_cores)

    # xt is pre-tiled on host: [MCH, 128, KO*mc] so each m-chunk is one fully
    # contiguous DMA.
    xt = nc.dram_tensor("xt", [MCH, P, KO * mc], BF16, kind="ExternalInput")
    wt = nc.dram_tensor("wt", [K, N], F32, kind="ExternalInput")
    sc = nc.dram_tensor("sc", [1, 2], F32, kind="ExternalInput")  # [alpha, r]
    y = nc.dram_tensor("y", [M, N], F32, kind="ExternalOutput")

    KP = KO // 2
    wp_view = wt.ap().rearrange("(kp j p) n -> kp p j n", p=P, j=2)
    y_view = y.ap().rearrange("(mo p) n -> mo p n", p=P)     # [M/128, 128, N]

    with tile.TileContext(nc) as tc:
        with (
            tc.tile_pool(name="scal", bufs=1) as scal,
            tc.tile_pool(name="wstage", bufs=3) as wstage,
            tc.tile_pool(name="tpool", bufs=1) as tpool,
            tc.tile_pool(name="xstage", bufs=2) as xstage,
            tc.tile_pool(name="outp", bufs=2) as outp,
            tc.tile_pool(name="psum", bufs=1, space="PSUM") as psum,
        ):
            sc_b = scal.tile([P, 2], F32)
            nc.sync.dma_start(out=sc_b, in_=sc.ap().to_broadcast([P, 2]))
            alpha = sc_b[:, 0:1]
            r = sc_b[:, 1:2]

            # ---------- quantize w -> t (bf16 ternary) ----------
            assert NQ == 1
            t = tpool.tile([P, KO, N], BF16)
            for kp in range(KP):
                par = kp % 2
                w_t = wstage.tile([P, 2, qw], F32, tag=f"wst{par}",
                                  bufs=2, name=f"wq_{kp}")
                nc.sync.dma_start(out=w_t, in_=wp_view[kp])
                if par == 0:
                    nc.vector.tensor_scalar(out=w_t, in0=w_t,
                                            scalar1=r,
                                            scalar2=MAGIC,
                                            op0=mybir.AluOpType.mult,
                                            op1=mybir.AluOpType.add)
                else:
                    nc.scalar.activation(
                        out=w_t, in_=w_t,
                        func=mybir.ActivationFunctionType.Copy,
                        bias=MAGIC, scale=r)
                nc.vector.tensor_scalar(out=w_t, in0=w_t,
                                        scalar1=MAGIC + 1.0,
                                        scalar2=MAGIC - 1.0,
                                        op0=mybir.AluOpType.min,
                                        op1=mybir.AluOpType.max)
                nc.scalar.activation(
                    out=t[:, 2 * kp:2 * kp + 2, :], in_=w_t,
                    func=mybir.ActivationFunctionType.Copy,
                    bias=-MAGIC, scale=1.0)

            # ---------- y = (x @ t^T) * alpha ----------
            assert MS == 1
            # While quantization streams in (DMA-paced ~100us), the PE would
            # drain each t[k] in ~1us and stall. Interleave the first two
            # chunks' k-loops (using all 8 PSUM banks) so the PE gets 2x the
            # work per quantized k-tile during the ramp.
            def evict(m_idx, psums):
                out_t = outp.tile([P, N], F32, tag="out", bufs=2,
                                  name=f"out_{m_idx}")
                for n in range(NT):
                    nc.vector.tensor_scalar_mul(
                        out_t[:, n * 512:(n + 1) * 512], psums[n], alpha)
                nc.sync.dma_start(out=y_view[m_idx], in_=out_t)

            def load_chunk(mch):
                xt_t = xstage.tile([P, KO, mc], BF16, tag="xt", bufs=2,
                                   name=f"xt_{mch}")
                nc.sync.dma_start(
                    out=xt_t,
                    in_=xt.ap()[mch].rearrange("p (ko m) -> p ko m", ko=KO))
                return xt_t

            def alloc_psums(m_idx):
                return [psum.tile([P, 512], F32, tag=f"mm{n}", bufs=2,
                                  name=f"ps_{m_idx}_{n}") for n in range(NT)]

            xt0, xt1 = load_chunk(0), load_chunk(1)
            ps0, ps1 = alloc_psums(0), alloc_psums(1)
            for kq in range(KO):
                for xt_t, psums in ((xt0, ps0), (xt1, ps1)):
                    lhs = xt_t[:, kq, :]
                    for n in range(NT):
                        nc.tensor.matmul(
                            psums[n], lhsT=lhs,
                            rhs=t[:, kq, n * 512:(n + 1) * 512],
                            start=(kq == 0), stop=(kq == KO - 1))
            evict(0, ps0)
            evict(1, ps1)

            for mch in range(2, MCH):
                xt_t = load_chunk(mch)
                psums = alloc_psums(mch)
                for kq in range(KO):
                    lhs = xt_t[:, kq, :]
                    for n in range(NT):
                        nc.tensor.matmul(
                            psums[n], lhsT=lhs,
                            rhs=t[:, kq, n * 512:(n + 1) * 512],
                            start=(kq == 0), stop=(kq == KO - 1))
                evict(mch, psums)

    nc.compile()
    return nc


def prep_xt(x_flat: np.ndarray, mc: int) -> np.ndarray:
    """Pre-tile x for contiguous chunk DMA:
    xt[mch, p, ko*mc+m] = bf16(x_flat[mch*mc+m, ko*128+p])."""
    m, k = x_flat.shape
    mch, ko = m // mc, k // P
    return np.ascontiguousarray(
        x_flat.reshape(mch, mc, ko, P).astype(ml_dtypes.bfloat16)
        .transpose(0, 3, 2, 1)).reshape(mch, P, ko * mc)


def kernel(x: np.ndarray, weight: np.ndarray) -> np.ndarray:
    assert x.shape == (B, S, K) and weight.shape == (N_FULL, K)
    core_ids = list(range(N_CORES))

    wts = [np.ascontiguousarray(weight[c * N:(c + 1) * N, :].T
                                .astype(np.float32)) for c in range(N_CORES)]

    # launch 1: per-shard |w| sums on device
    nc1 = build_reduce_nc(K, N)
    res1 = run_bass_kernel_spmd(nc1, [{"wt": w} for w in wts], core_ids)
    partials = [np.float32(res1.results[c]["s"][0, 0]) for c in range(N_CORES)]

    # the all-reduce across shards: 8 scalar adds
    s_tot = np.float32(0.0)
    for p in partials:
        s_tot = np.float32(s_tot + p)
    alpha = np.float32(s_tot * np.float32(MEAN_SCALE))
    r = np.float32(np.float32(1.0) / np.float32(alpha + np.float32(EPS)))
    sc = np.array([[alpha, r]], dtype=np.float32)

    # launch 2: quantize + matmul
    xt = prep_xt(x.reshape(M, K), 128)
    nc2 = build_main_nc(M, K, N)
    in_maps = [{"xt": xt, "wt": wts[c], "sc": sc} for c in range(N_CORES)]
    res2 = run_bass_kernel_spmd(nc2, in_maps, core_ids)

    y = np.concatenate([res2.results[c]["y"] for c in range(N_CORES)], axis=1)
    return np.ascontiguousarray(y.reshape(B, S, N_FULL).astype(np.float32))
